# revision 34
# baseline (speedup 1.0000x reference)
"""Trainium2 Bass kernel for nn_Custom_Decoder (segment_reduce).

Reference computation:
    est = einsum('bcle,we->bclw', dec_input[8,2,3999,512], weight[16,512])
    out = overlap_and_add(est, frame_step=8)   -> [8, 2, 32000]

Strategy (pure data parallel, one batch element per NeuronCore):
  - Host prep (layout only): per core b, xt = dec_input[b].transpose -> [2, 512, 3999]
    so that E sits on SBUF partitions (contraction dim), rows on the free dim.
    wt = weight.T -> [512, 16].
  - On-chip per (c, row-chunk of 512 frames): 4 PSUM-accumulated matmuls
    (lhsT = weight chunk [128e, 16w] stationary, rhs = x chunk [128e, F rows])
    -> psum [16w, F] holding est.T for the chunk.
  - Overlap-add (frame_step=8, frame_len=16): out[j, r] = est[j, r] + est[j+8, r-1]
    done with one partition-offset DVE add per chunk + tiny seam fixups.
  - Output stored j-major [2, 8, 4000]; host permutes to [2, 32000] (layout only).
"""

import numpy as np

B, C, FRAMES, E, W = 8, 2, 3999, 512, 16
STEP = W // 2  # 8
T_OUT = STEP * (FRAMES - 1) + W  # 32000
ROWS = T_OUT // STEP  # 4000
CHUNK = 512
NCHUNKS = (FRAMES + CHUNK - 1) // CHUNK  # 8 (last chunk = 415 frames)
# fp32r matmuls need an even moving free-dim; pad frames 3999 -> 4000 with a
# zero column so the last chunk can run 416 wide.
FRAMES_PAD = FRAMES + 1  # 4000
PW = 40  # weight padded to 40 cols: A half -> psum part 0..7, B half -> 32..39
NEC = E // 128  # 4 e-chunks
N_CORES = 8

_NC_CACHE = {}


def _build_nc(mode="f32r"):
    import concourse.mybir as mybir
    import concourse.tile as tile
    from concourse import bacc

    f32 = mybir.dt.float32
    dt_in = {
        "f32r": mybir.dt.float32r,
        "f32": mybir.dt.float32,
        "f16": mybir.dt.float16,
        "bf16": mybir.dt.bfloat16,
    }[mode]

    nc = bacc.Bacc("TRN2", target_bir_lowering=False, debug=False)

    xt = nc.dram_tensor("xt", [C, E, FRAMES_PAD], dt_in, kind="ExternalInput")
    # weight pre-padded on host: [E, PW] with zeros in cols 8..31
    wt = nc.dram_tensor("wt", [E, PW], dt_in, kind="ExternalInput")
    out = nc.dram_tensor("out", [C, STEP, ROWS], f32, kind="ExternalOutput")

    # input DMA granularity: quarters of the frame axis (2 compute chunks)
    QW = 2 * CHUNK  # 1024
    NQ = (FRAMES_PAD + QW - 1) // QW  # 4 (last quarter = 928 cols)

    with tile.TileContext(nc) as tc:
        with (
            tc.tile_pool(name="xpool", bufs=C * NQ) as xpool,
            tc.tile_pool(name="wpool", bufs=1) as wpool,
            tc.tile_pool(name="opool", bufs=1) as opool,
            tc.tile_pool(name="bpool", bufs=6) as bpool,
            tc.tile_pool(name="pspool", bufs=8, space="PSUM") as pspool,
        ):
            # padded weight chunks side by side (zeros baked in on host);
            # one 3D-AP DMA: HWDGE issue is ~0.6us of (shared) sequencer
            # time per dma_start, so merge aggressively
            wt_sb = wpool.tile([128, NEC * PW], dt_in)
            nc.sync.dma_start(
                wt_sb[:].rearrange("p (ec w) -> p ec w", ec=NEC),
                wt[:].rearrange("(ec p) w -> p ec w", p=128))

            # j-major output, spread over 4 partition quadrants so output
            # DMAs hit 4 SDMA engine-groups instead of 1: region for (c, q)
            # sits at partition base OPB[c][q%2], sb cols [(q//2)*QW, ...).
            OPB = [[0, 64], [32, 96]]
            out_sb = opool.tile([128, (NQ // 2) * QW], f32)

            # x quarter tiles: xq[c][q] = [128, NEC, <=1024]: all 4 e-chunks
            # of one frame-quarter in ONE 3D-AP DMA (HWDGE issue time is
            # shared across SP/ACT queues, so fewer dma_starts beat
            # queue-splitting). The last quarter is split into two pieces so
            # the final chunk's matmuls start during the last transfer.
            xq = [[None] * NQ for _ in range(C)]
            for c in range(C):
                for q in range(NQ):
                    x_sb = xpool.tile([128, NEC * QW], dt_in,
                                      name=f"x_{c}_{q}", tag="x")
                    if q < NQ - 1:
                        nc.sync.dma_start(
                            x_sb[:].rearrange(
                                "p (ec f) -> p ec f", ec=NEC),
                            xt[c].rearrange("(ec p) f -> p ec f",
                                            p=128)[:, :, q * QW:(q + 1) * QW])
                    else:
                        nc.sync.dma_start(
                            x_sb[:].rearrange(
                                "p (ec f) -> p ec f", ec=NEC)[:, :, :CHUNK],
                            xt[c].rearrange(
                                "(ec p) f -> p ec f",
                                p=128)[:, :, q * QW:q * QW + CHUNK])
                        tail_w = FRAMES_PAD - q * QW - CHUNK  # 416
                        nc.sync.dma_start(
                            x_sb[:].rearrange(
                                "p (ec f) -> p ec f",
                                ec=NEC)[:, :, CHUNK:CHUNK + tail_w],
                            xt[c].rearrange(
                                "(ec p) f -> p ec f",
                                p=128)[:, :, q * QW + CHUNK:FRAMES_PAD])
                    xq[c][q] = x_sb

            for c in range(C):
                b_prev = None
                ft_prev = None
                for t in range(NCHUNKS):
                    f0 = t * CHUNK
                    q = t // 2
                    fq = (t % 2) * CHUNK  # offset inside the quarter tile
                    pb = OPB[c][q % 2]  # out region partition base
                    cb = (q // 2) * QW + fq  # out region col base of chunk
                    ft = min(CHUNK, FRAMES - f0)  # real frames in this chunk
                    ftm = ft + (ft % 2)  # even moving free-dim (fp32r rule)
                    ps = pspool.tile([PW, CHUNK], f32, name="ps")
                    for ec in range(NEC):
                        nc.tensor.matmul(
                            ps[:, :ftm],
                            wt_sb[:, ec * PW:(ec + 1) * PW],
                            xq[c][q][:, ec * QW + fq:ec * QW + fq + ftm],
                            start=(ec == 0),
                            stop=(ec == NEC - 1),
                        )
                    # B half -> SBUF (the HW forbids two PSUM reads in one
                    # op); on ACT, which carries no DMA issues or other work.
                    # Last chunk: DVE, avoiding the extra cross-engine hop on
                    # the critical tail.
                    b_sb = bpool.tile([STEP, CHUNK], f32, name=f"b_{c}_{t}",
                                      tag="b")
                    if t == NCHUNKS - 1:
                        nc.vector.tensor_copy(b_sb[:, :ft],
                                              ps[32:32 + STEP, :ft])
                    else:
                        nc.scalar.copy(b_sb[:, :ft], ps[32:32 + STEP, :ft])
                    # bulk: out[j, f0+1 : f0+ft] = A[1:ft] + B[0:ft-1]
                    nc.vector.tensor_add(
                        out_sb[pb:pb + STEP, cb + 1:cb + ft],
                        ps[0:STEP, 1:ft],
                        b_sb[:, 0:ft - 1],
                    )
                    # seam column r = f0: A[0] (+ B_prev[last])
                    if t == 0:
                        nc.vector.tensor_copy(
                            out_sb[pb:pb + STEP, cb:cb + 1],
                            ps[0:STEP, 0:1],
                        )
                    else:
                        nc.vector.tensor_add(
                            out_sb[pb:pb + STEP, cb:cb + 1],
                            ps[0:STEP, 0:1],
                            b_prev[:, ft_prev - 1:ft_prev],
                        )
                    b_prev, ft_prev = b_sb, ft
                    # after odd chunk t, output quarter q is final; stream it
                    # out on the (idle) SWDGE queue
                    if t % 2 == 1 and t < NCHUNKS - 1:
                        nc.gpsimd.dma_start(
                            out[c, :, q * QW:(q + 1) * QW],
                            out_sb[pb:pb + STEP,
                                   (q // 2) * QW:(q // 2) * QW + QW])
                    # after chunk NCHUNKS-2, cols [f0, f0+CHUNK) of the last
                    # quarter are final except the seam written by the next
                    # chunk; ship cols [q*QW, q*QW+CHUNK) minus nothing —
                    # seam col (t+1)*CHUNK is outside this span
                    if t == NCHUNKS - 2:
                        nc.gpsimd.dma_start(
                            out[c, :, q * QW:q * QW + CHUNK],
                            out_sb[pb:pb + STEP,
                                   (q // 2) * QW:(q // 2) * QW + CHUNK])
                # final column r = ROWS-1 = B[last frame]
                qlast = (NCHUNKS - 1) // 2
                pb = OPB[c][qlast % 2]
                nc.vector.tensor_copy(
                    out_sb[pb:pb + STEP,
                           (qlast // 2) * QW + ROWS - 1 - qlast * QW:
                           (qlast // 2) * QW + ROWS - qlast * QW],
                    b_prev[:, ft_prev - 1:ft_prev],
                )
                # trailing piece of the last quarter (cols incl. final col)
                nc.gpsimd.dma_start(
                    out[c, :, qlast * QW + CHUNK:ROWS],
                    out_sb[pb:pb + STEP,
                           (qlast // 2) * QW + CHUNK:(qlast // 2) * QW + ROWS
                           - qlast * QW])

    nc.compile()
    return nc


def get_nc(mode="f16"):
    if mode not in _NC_CACHE:
        _NC_CACHE[mode] = _build_nc(mode)
    return _NC_CACHE[mode]


def prep_core_inputs(dec_input, weight, mode="f16"):
    """Host-side layout prep + shard: one batch element per core."""
    np_dt = {"f32r": np.float32, "f32": np.float32,
             "f16": np.float16, "bf16": None}[mode]
    if np_dt is None:
        import ml_dtypes
        np_dt = ml_dtypes.bfloat16
    wt = np.zeros((E, PW), dtype=np_dt)
    wT = weight.T.astype(np_dt)  # [512, 16]
    wt[:, 0:STEP] = wT[:, 0:STEP]
    wt[:, 32:PW] = wT[:, STEP:W]
    in_maps = []
    for b in range(N_CORES):
        xt = np.zeros((C, E, FRAMES_PAD), dtype=np_dt)
        xt[:, :, :FRAMES] = dec_input[b].astype(np_dt).transpose(0, 2, 1)
        in_maps.append({"xt": xt, "wt": wt})
    return in_maps


def assemble_output(core_outs):
    """core_outs[b]: [C, STEP, ROWS] j-major -> full [B, C, T_OUT]."""
    full = np.empty((B, C, T_OUT), dtype=np.float32)
    for b in range(N_CORES):
        o = np.asarray(core_outs[b])  # [C, 8, 4000]
        full[b] = o.transpose(0, 2, 1).reshape(C, T_OUT)
    return full


def run_on_hw(dec_input, weight, mode="f16", trace=False):
    from concourse.bass_utils import run_bass_kernel_spmd

    nc = get_nc(mode)
    in_maps = prep_core_inputs(dec_input, weight, mode)
    res = run_bass_kernel_spmd(nc, in_maps, core_ids=list(range(N_CORES)),
                               trace=trace)
    outs = [res.results[b]["out"] for b in range(N_CORES)]
    return assemble_output(outs), res


def kernel(dec_input, weight):
    out, _ = run_on_hw(np.asarray(dec_input), np.asarray(weight))
    return out


# revision 35
# speedup vs baseline: 1.1034x; 1.1034x over previous
"""Trainium2 Bass kernel for nn_Custom_Decoder (segment_reduce).

Reference computation:
    est = einsum('bcle,we->bclw', dec_input[8,2,3999,512], weight[16,512])
    out = overlap_and_add(est, frame_step=8)   -> [8, 2, 32000]

Strategy (pure data parallel, one batch element per NeuronCore):
  - Host prep (layout only): per core b, xt = dec_input[b].transpose -> [2, 512, 3999]
    so that E sits on SBUF partitions (contraction dim), rows on the free dim.
    wt = weight.T -> [512, 16].
  - On-chip per (c, row-chunk of 512 frames): 4 PSUM-accumulated matmuls
    (lhsT = weight chunk [128e, 16w] stationary, rhs = x chunk [128e, F rows])
    -> psum [16w, F] holding est.T for the chunk.
  - Overlap-add (frame_step=8, frame_len=16): out[j, r] = est[j, r] + est[j+8, r-1]
    done with one partition-offset DVE add per chunk + tiny seam fixups.
  - Output stored j-major [2, 8, 4000]; host permutes to [2, 32000] (layout only).
"""

import numpy as np

B, C, FRAMES, E, W = 8, 2, 3999, 512, 16
STEP = W // 2  # 8
T_OUT = STEP * (FRAMES - 1) + W  # 32000
ROWS = T_OUT // STEP  # 4000
CHUNK = 512
NCHUNKS = (FRAMES + CHUNK - 1) // CHUNK  # 8 (last chunk = 415 frames)
# fp32r matmuls need an even moving free-dim; pad frames 3999 -> 4000 with a
# zero column so the last chunk can run 416 wide.
FRAMES_PAD = FRAMES + 1  # 4000
PW = 40  # weight padded to 40 cols: A half -> psum part 0..7, B half -> 32..39
NEC = E // 128  # 4 e-chunks
N_CORES = 8

_NC_CACHE = {}


def _build_nc(mode="f32r"):
    import concourse.mybir as mybir
    import concourse.tile as tile
    from concourse import bacc

    f32 = mybir.dt.float32
    dt_in = {
        "f32r": mybir.dt.float32r,
        "f32": mybir.dt.float32,
        "f16": mybir.dt.float16,
        "bf16": mybir.dt.bfloat16,
    }[mode]

    nc = bacc.Bacc("TRN2", target_bir_lowering=False, debug=False)

    xt = nc.dram_tensor("xt", [C, E, FRAMES_PAD], dt_in, kind="ExternalInput")
    # weight pre-padded on host: [E, PW] with zeros in cols 8..31
    wt = nc.dram_tensor("wt", [E, PW], dt_in, kind="ExternalInput")
    out = nc.dram_tensor("out", [C, STEP, ROWS], f32, kind="ExternalOutput")

    # input DMA granularity: quarters of the frame axis (2 compute chunks)
    QW = 2 * CHUNK  # 1024
    NQ = (FRAMES_PAD + QW - 1) // QW  # 4 (last quarter = 928 cols)

    with tile.TileContext(nc) as tc:
        with (
            tc.tile_pool(name="xpool", bufs=C * NQ) as xpool,
            tc.tile_pool(name="wpool", bufs=1) as wpool,
            tc.tile_pool(name="opool", bufs=1) as opool,
            tc.tile_pool(name="bpool", bufs=6) as bpool,
            tc.tile_pool(name="pspool", bufs=8, space="PSUM") as pspool,
        ):
            # padded weight chunks side by side (zeros baked in on host);
            # one 3D-AP DMA: HWDGE issue is ~0.6us of (shared) sequencer
            # time per dma_start, so merge aggressively
            wt_sb = wpool.tile([128, NEC * PW], dt_in)

            # j-major output, spread over 4 partition quadrants so output
            # DMAs hit 4 SDMA engine-groups instead of 1: region for (c, q)
            # sits at partition base OPB[c][q%2], sb cols [(q//2)*QW, ...).
            OPB = [[0, 64], [32, 96]]
            out_sb = opool.tile([128, (NQ // 2) * QW], f32)

            # x quarter tiles: xq[c][q] = [128, NEC, <=1024]: all 4 e-chunks
            # of one frame-quarter in ONE 3D-AP DMA (HWDGE issue time is
            # shared across SP/ACT queues, so fewer dma_starts beat
            # queue-splitting). The last quarter is split into two pieces so
            # the final chunk's matmuls start during the last transfer.
            xq = [[None] * NQ for _ in range(C)]
            for c in range(C):
                for q in range(NQ):
                    x_sb = xpool.tile([128, NEC * QW], dt_in,
                                      name=f"x_{c}_{q}", tag="x")
                    if q < NQ - 1:
                        nc.sync.dma_start(
                            x_sb[:].rearrange(
                                "p (ec f) -> p ec f", ec=NEC),
                            xt[c].rearrange("(ec p) f -> p ec f",
                                            p=128)[:, :, q * QW:(q + 1) * QW])
                    else:
                        nc.sync.dma_start(
                            x_sb[:].rearrange(
                                "p (ec f) -> p ec f", ec=NEC)[:, :, :CHUNK],
                            xt[c].rearrange(
                                "(ec p) f -> p ec f",
                                p=128)[:, :, q * QW:q * QW + CHUNK])
                        tail_w = FRAMES_PAD - q * QW - CHUNK  # 416
                        nc.sync.dma_start(
                            x_sb[:].rearrange(
                                "p (ec f) -> p ec f",
                                ec=NEC)[:, :, CHUNK:CHUNK + tail_w],
                            xt[c].rearrange(
                                "(ec p) f -> p ec f",
                                p=128)[:, :, q * QW + CHUNK:FRAMES_PAD])
                    xq[c][q] = x_sb
                    if c == 0 and q == 0:
                        # weight load issued second: it is tiny and still
                        # lands long before the first matmul, while the
                        # first x transfer's HBM window starts ~0.8us sooner
                        nc.sync.dma_start(
                            wt_sb[:].rearrange("p (ec w) -> p ec w", ec=NEC),
                            wt[:].rearrange("(ec p) w -> p ec w", p=128))

            for c in range(C):
                b_prev = None
                ft_prev = None
                for t in range(NCHUNKS):
                    f0 = t * CHUNK
                    q = t // 2
                    fq = (t % 2) * CHUNK  # offset inside the quarter tile
                    pb = OPB[c][q % 2]  # out region partition base
                    cb = (q // 2) * QW + fq  # out region col base of chunk
                    ft = min(CHUNK, FRAMES - f0)  # real frames in this chunk
                    ftm = ft + (ft % 2)  # even moving free-dim (fp32r rule)
                    ps = pspool.tile([PW, CHUNK], f32, name="ps")
                    for ec in range(NEC):
                        nc.tensor.matmul(
                            ps[:, :ftm],
                            wt_sb[:, ec * PW:(ec + 1) * PW],
                            xq[c][q][:, ec * QW + fq:ec * QW + fq + ftm],
                            start=(ec == 0),
                            stop=(ec == NEC - 1),
                        )
                    # B half -> SBUF (the HW forbids two PSUM reads in one
                    # op); on ACT, which carries no DMA issues or other work.
                    # Last chunk: DVE, avoiding the extra cross-engine hop on
                    # the critical tail.
                    b_sb = bpool.tile([STEP, CHUNK], f32, name=f"b_{c}_{t}",
                                      tag="b")
                    if t == NCHUNKS - 1:
                        nc.vector.tensor_copy(b_sb[:, :ft],
                                              ps[32:32 + STEP, :ft])
                    else:
                        nc.scalar.copy(b_sb[:, :ft], ps[32:32 + STEP, :ft])
                    # bulk: out[j, f0+1 : f0+ft] = A[1:ft] + B[0:ft-1]
                    nc.vector.tensor_add(
                        out_sb[pb:pb + STEP, cb + 1:cb + ft],
                        ps[0:STEP, 1:ft],
                        b_sb[:, 0:ft - 1],
                    )
                    # seam column r = f0: A[0] (+ B_prev[last])
                    if t == 0:
                        nc.vector.tensor_copy(
                            out_sb[pb:pb + STEP, cb:cb + 1],
                            ps[0:STEP, 0:1],
                        )
                    else:
                        nc.vector.tensor_add(
                            out_sb[pb:pb + STEP, cb:cb + 1],
                            ps[0:STEP, 0:1],
                            b_prev[:, ft_prev - 1:ft_prev],
                        )
                    b_prev, ft_prev = b_sb, ft
                    # after odd chunk t, output quarter q is final; stream it
                    # out on the (idle) SWDGE queue
                    if t % 2 == 1 and t < NCHUNKS - 1:
                        nc.gpsimd.dma_start(
                            out[c, :, q * QW:(q + 1) * QW],
                            out_sb[pb:pb + STEP,
                                   (q // 2) * QW:(q // 2) * QW + QW])
                    # after chunk NCHUNKS-2, cols [f0, f0+CHUNK) of the last
                    # quarter are final except the seam written by the next
                    # chunk; ship cols [q*QW, q*QW+CHUNK) minus nothing —
                    # seam col (t+1)*CHUNK is outside this span
                    if t == NCHUNKS - 2:
                        nc.gpsimd.dma_start(
                            out[c, :, q * QW:q * QW + CHUNK],
                            out_sb[pb:pb + STEP,
                                   (q // 2) * QW:(q // 2) * QW + CHUNK])
                # final column r = ROWS-1 = B[last frame]
                qlast = (NCHUNKS - 1) // 2
                pb = OPB[c][qlast % 2]
                nc.vector.tensor_copy(
                    out_sb[pb:pb + STEP,
                           (qlast // 2) * QW + ROWS - 1 - qlast * QW:
                           (qlast // 2) * QW + ROWS - qlast * QW],
                    b_prev[:, ft_prev - 1:ft_prev],
                )
                # trailing piece of the last quarter (cols incl. final col)
                # on the long-idle SP HWDGE queue: lower first-byte latency
                nc.sync.dma_start(
                    out[c, :, qlast * QW + CHUNK:ROWS],
                    out_sb[pb:pb + STEP,
                           (qlast // 2) * QW + CHUNK:(qlast // 2) * QW + ROWS
                           - qlast * QW])

    nc.compile()
    return nc


def get_nc(mode="f16"):
    if mode not in _NC_CACHE:
        _NC_CACHE[mode] = _build_nc(mode)
    return _NC_CACHE[mode]


def prep_core_inputs(dec_input, weight, mode="f16"):
    """Host-side layout prep + shard: one batch element per core."""
    np_dt = {"f32r": np.float32, "f32": np.float32,
             "f16": np.float16, "bf16": None}[mode]
    if np_dt is None:
        import ml_dtypes
        np_dt = ml_dtypes.bfloat16
    wt = np.zeros((E, PW), dtype=np_dt)
    wT = weight.T.astype(np_dt)  # [512, 16]
    wt[:, 0:STEP] = wT[:, 0:STEP]
    wt[:, 32:PW] = wT[:, STEP:W]
    in_maps = []
    for b in range(N_CORES):
        xt = np.zeros((C, E, FRAMES_PAD), dtype=np_dt)
        xt[:, :, :FRAMES] = dec_input[b].astype(np_dt).transpose(0, 2, 1)
        in_maps.append({"xt": xt, "wt": wt})
    return in_maps


def assemble_output(core_outs):
    """core_outs[b]: [C, STEP, ROWS] j-major -> full [B, C, T_OUT]."""
    full = np.empty((B, C, T_OUT), dtype=np.float32)
    for b in range(N_CORES):
        o = np.asarray(core_outs[b])  # [C, 8, 4000]
        full[b] = o.transpose(0, 2, 1).reshape(C, T_OUT)
    return full


def run_on_hw(dec_input, weight, mode="f16", trace=False):
    from concourse.bass_utils import run_bass_kernel_spmd

    nc = get_nc(mode)
    in_maps = prep_core_inputs(dec_input, weight, mode)
    res = run_bass_kernel_spmd(nc, in_maps, core_ids=list(range(N_CORES)),
                               trace=trace)
    outs = [res.results[b]["out"] for b in range(N_CORES)]
    return assemble_output(outs), res


def kernel(dec_input, weight):
    out, _ = run_on_hw(np.asarray(dec_input), np.asarray(weight))
    return out
